# revision 1
# baseline (speedup 1.0000x reference)
"""Trainium2 Bass kernel for nn_MixedLinear_89979564851799.

The reference computes y = x @ W.T where W is the block-dequantized weight
(fp4 partition: per-16 e4m3 inner scale x per-128 fp32 outer scale; fp8
partition: per-32 e8m0 scale).  setup_inputs() also ships the module's
precomputed dequantized transposed weight buffer w_t (IN, OUT), built with
the exact same multiply ordering, so y == x @ w_t bit-for-bit up to fp32
matmul reassociation.  The kernel therefore runs a single 8192x4096x4096
matmul, data-parallel over tokens across 8 NeuronCores, with bf16 operands
and fp32 PSUM accumulation (measured rel err ~2.3e-3 vs the fp32 reference).

Host-side preprocessing (not on the HW critical path): transpose x to
[IN, TOKENS] (the PE needs the contraction dim on partitions for both
operands) and cast both operands to bf16.

Per-core kernel (M=1024 tokens, K=4096, N=4096):
  - x^T tile [128, 32kt, 1024] bf16 stays resident in SBUF (64KB/part)
  - stream w_t in 8 chunks of 512 cols, double-buffered
  - for each (n-chunk, m-tile): accumulate 32 k-tile matmuls into one
    PSUM bank (8 banks = 8 m-tiles in flight), copy back on DVE, DMA out
"""

import os
import numpy as np
import ml_dtypes

P = 128
TOKENS, IN, OUT = 8192, 4096, 4096
NCORES = 8
M_PER_CORE = TOKENS // NCORES      # 1024
KT = IN // P                       # 32 k-tiles
MT = M_PER_CORE // P               # 8 m-tiles
NCH = 8                            # n chunks
NW = OUT // NCH                    # 512 cols per chunk (= 1 PSUM bank fp32)
KG = 4                             # k-groups per n-chunk load (DMA granularity)
KTG = KT // KG                     # 8 k-tiles per group

# Results of the traced run (exec_time_ns etc.) for test harnesses.
LAST_RESULT = None
_BUILT = None


def _patch_tile_drain():
    """The walrus build in this container rejects instructions carrying more
    than one sync-wait (CoreV3GenImpl setupSyncWait: "Too many sync wait
    commands").  Tile's scheduler freely assigns several waits to one
    instruction, so (a) wrap _commit_instruction to hoist extra waits onto
    single-wait NOPs on the same engine just before the offender, and
    (b) split the kernel-tail Drain (which collects one wait per DMA queue)
    into a chain of single-wait Drains."""
    import concourse.tile as tile_mod
    import concourse.mybir as mybir
    import bass_rust
    from concourse.vector_clock import ScopedClock

    if getattr(tile_mod.TileContext, "_single_wait_drain_patch", False):
        return

    orig_commit = tile_mod.TileContext._commit_instruction

    def _commit_instruction(self, inst, lazy_reg_writes=True):
        si = getattr(inst, "sync_info", None)
        if (
            si is not None
            and len(si.on_wait) > 1
            and inst.engine != mybir.EngineType.Unassigned
        ):
            waits = list(si.on_wait)
            for w in waits[:-1]:
                nop = mybir.InstNoOp(
                    name=self.nc.get_next_instruction_name(),
                    engine=inst.engine,
                    sync_info=mybir.SyncInfo(on_wait=[w], on_update=[]),
                    bass_nofuse=True,
                )
                orig_commit(self, nop, lazy_reg_writes=False)
            inst.sync_info = mybir.SyncInfo(
                on_wait=[waits[-1]], on_update=list(si.on_update)
            )
        return orig_commit(self, inst, lazy_reg_writes)

    tile_mod.TileContext._commit_instruction = _commit_instruction

    def _drain_and_barrier(self, tick_clock, wait_clock):
        drain_inst = self.nc.sync.drain()
        wait_clock.add_sem_waits(
            drain_inst.ins, ScopedClock({None: tick_clock.global_clock})
        )
        si = drain_inst.ins.sync_info
        if si is not None and len(si.on_wait) > 1:
            waits = list(si.on_wait)
            drain_inst.ins.sync_info = bass_rust.SyncInfo(
                on_wait=[waits[0]], on_update=list(si.on_update)
            )
            for w in waits[1:]:
                extra = self.nc.sync.drain()
                extra.ins.sync_info = bass_rust.SyncInfo(on_wait=[w], on_update=[])
        self.nc.all_engine_barrier()
        popped = self.nc._tile_sem_poison_stack.pop()
        assert popped is self._sem_poison
        self.nc.clear_and_free_semaphores(list(self.sems.allocated().values()))
        self.nc.all_engine_barrier()

    tile_mod.TileContext._drain_and_barrier = _drain_and_barrier
    tile_mod.TileContext._single_wait_drain_patch = True


def _build():
    global _BUILT
    if _BUILT is not None:
        return _BUILT
    import concourse.bass as bass
    import concourse.tile as tile
    from concourse import mybir

    _patch_tile_drain()

    nc = bass.Bass("TRN2", debug=False)
    # xt is pre-tiled on the host: [mt][p(k)][kt][m] so each per-mt DMA
    # reads 8KB contiguous per partition line.
    xt_d = nc.dram_tensor(
        "xt", [MT, P, KT, P], mybir.dt.bfloat16, kind="ExternalInput"
    ).ap()
    # w is pre-tiled on the host too: [nch][kg][p(k)][ktg][n] so each
    # (nch, kg) DMA reads 8KB contiguous per partition line.
    w_d = nc.dram_tensor(
        "w", [NCH, KG, P, KTG, NW], mybir.dt.bfloat16, kind="ExternalInput"
    ).ap()
    y_d = nc.dram_tensor(
        "y", [M_PER_CORE, OUT], mybir.dt.float32, kind="ExternalOutput"
    ).ap()

    with tile.TileContext(nc) as tc:
        with (
            tc.tile_pool(name="xt", bufs=1) as xt_pool,
            tc.tile_pool(name="w", bufs=3) as w_pool,
            tc.tile_pool(name="y", bufs=8) as y_pool,
            tc.tile_pool(name="ps", bufs=8, space="PSUM") as ps_pool,
        ):
            # x^T resident in SBUF, one tile per m-tile so the first
            # matmuls only wait for their own 1MB slice.
            xt_sbs = [None] * MT

            def load_xt(mt, split=1):
                xt_sb = xt_pool.tile([P, KT, P], mybir.dt.bfloat16, tag=f"xt{mt}")
                step = KT // split
                for s in range(split):
                    nc.sync.dma_start(
                        xt_sb[:, s * step : (s + 1) * step, :],
                        xt_d[mt, :, s * step : (s + 1) * step, :],
                    )
                xt_sbs[mt] = xt_sb

            def load_w_kg(nch, kg, split=1):
                w_sb = w_pool.tile([P, KTG, NW], mybir.dt.bfloat16, tag=f"w{kg}")
                step = KTG // split
                for s in range(split):
                    nc.sync.dma_start(
                        w_sb[:, s * step : (s + 1) * step, :],
                        w_d[nch, kg, :, s * step : (s + 1) * step, :],
                    )
                return w_sb

            # Head ordering: the first matmul needs only xt[0] + w[0,kg0]
            # (2MB), so emit those first (split across queues), then the
            # rest of chunk 0 and the remaining xt slices.
            load_xt(0, split=4)
            w_sbs0 = [load_w_kg(0, 0, split=4)]
            for kg in range(1, KG):
                w_sbs0.append(load_w_kg(0, kg, split=2))
            for mt in range(1, MT):
                load_xt(mt)

            def lhsT(mt, kt):
                return xt_sbs[mt][:, kt, :]

            for nch in range(NCH):
                w_sbs = w_sbs0 if nch == 0 else [load_w_kg(nch, kg) for kg in range(KG)]
                for mt in range(MT):
                    ps = ps_pool.tile([P, NW], mybir.dt.float32)
                    for kt in range(KT):
                        nc.tensor.matmul(
                            ps[:],
                            lhsT=lhsT(mt, kt),
                            rhs=w_sbs[kt // KTG][:, kt % KTG, :],
                            start=(kt == 0),
                            stop=(kt == KT - 1),
                        )
                    y_sb = y_pool.tile([P, NW], mybir.dt.float32)
                    nc.vector.tensor_copy(y_sb[:], ps[:])
                    half = NW // 2
                    for s in range(2):
                        nc.scalar.dma_start(
                            y_d[
                                mt * P : (mt + 1) * P,
                                nch * NW + s * half : nch * NW + (s + 1) * half,
                            ],
                            y_sb[:, s * half : (s + 1) * half],
                        )
    _BUILT = nc
    return nc


def kernel(x, w_q_fp4, w_os_fp4, w_is_fp4, w_t, w_q_fp8, w_s_fp8):
    global LAST_RESULT
    from concourse.bass_utils import run_bass_kernel_spmd

    x = np.asarray(x, dtype=np.float32)
    w_t = np.asarray(w_t, dtype=np.float32)

    nc = _build()

    xt = np.ascontiguousarray(x.T).astype(ml_dtypes.bfloat16)  # [IN, TOKENS]
    w = w_t.astype(ml_dtypes.bfloat16)
    # [kg*KTG*P + ktg*P + p, nch*NW + n] -> [nch, kg, p, ktg, n]
    w_tiled = np.ascontiguousarray(
        w.reshape(KG, KTG, P, NCH, NW).transpose(3, 0, 2, 1, 4)
    )
    in_maps = []
    for i in range(NCORES):
        xc = xt[:, i * M_PER_CORE : (i + 1) * M_PER_CORE]  # [IN, M]
        # [kt*P, mt*P] -> [mt, p, kt, m]
        xc_t = np.ascontiguousarray(
            xc.reshape(KT, P, MT, P).transpose(2, 1, 0, 3)
        )
        in_maps.append({"xt": xc_t, "w": w_tiled})
    res = None
    for attempt in range(3):
        try:
            res = run_bass_kernel_spmd(
                nc,
                in_maps,
                list(range(NCORES)),
                trace=bool(os.environ.get("BASS_TRACE")),
            )
            break
        except Exception:
            # transient device errors (e.g. NRT_EXEC_UNIT_UNRECOVERABLE)
            # have been observed once and succeeded on retry
            if attempt == 2:
                raise
    LAST_RESULT = res
    return np.concatenate([res.results[i]["y"] for i in range(NCORES)], axis=0)



# revision 7
# speedup vs baseline: 1.0150x; 1.0150x over previous
"""Trainium2 Bass kernel for nn_MixedLinear_89979564851799.

The reference computes y = x @ W.T where W is the block-dequantized weight
(fp4 partition: per-16 e4m3 inner scale x per-128 fp32 outer scale; fp8
partition: per-32 e8m0 scale).  setup_inputs() also ships the module's
precomputed dequantized transposed weight buffer w_t (IN, OUT), built with
the exact same multiply ordering, so y == x @ w_t bit-for-bit up to fp32
matmul reassociation.  The kernel therefore runs a single 8192x4096x4096
matmul, data-parallel over tokens across 8 NeuronCores, with bf16 operands
and fp32 PSUM accumulation (measured rel err ~2.3e-3 vs the fp32 reference).

Host-side preprocessing (not on the HW critical path): transpose x to
[IN, TOKENS] (the PE needs the contraction dim on partitions for both
operands) and cast both operands to bf16.

Per-core kernel (M=1024 tokens, K=4096, N=4096), v2 layout tuned from the
ntff profile of v1 (471us: 15.9us head waiting on one serialized DMA
descriptor stream, ~16us of DMA-chase gaps in the first n-chunk, 6us tail):
  - x^T [128, 32kt, 1024m] bf16 resident in SBUF (64KB/part), loaded in
    32 per-k-tile descriptors interleaved with n-chunk-0's w descriptors
    so the first matmul can start after ~2 descriptors (~160KB landed)
  - n-chunk 0 runs k-OUTER across all 8 m-tiles (8 PSUM banks live) so
    per-k-tile DMA demand is 384KB/1.73us = 222GB/s < the ~300GB/s the
    16 DMA engines deliver: the PE never waits on w streaming
  - n-chunks 1-7 run m-inner (baseline style, staggered psum copies),
    w double-buffered in [128, 8kt, 512] chunks
  - psum -> sbuf copy on DVE, y DMA out on the scalar queue; the last
    m-group's copy/store is split across engines/queues to shrink the tail
"""

import os
import numpy as np
import ml_dtypes

P = 128
TOKENS, IN, OUT = 8192, 4096, 4096
NCORES = 8
M_PER_CORE = TOKENS // NCORES      # 1024
KT = IN // P                       # 32 k-tiles
MT = M_PER_CORE // P               # 8 m-tiles
NCH = 8                            # n chunks
NW = OUT // NCH                    # 512 cols per chunk (= 1 PSUM bank fp32)
KG = 4                             # k-groups per n-chunk load (DMA granularity)
KTG = KT // KG                     # 8 k-tiles per group

# Results of the traced run (exec_time_ns etc.) for test harnesses.
LAST_RESULT = None
_BUILT = None


def _patch_tile_drain():
    """The walrus build in this container rejects instructions carrying more
    than one sync-wait (CoreV3GenImpl setupSyncWait: "Too many sync wait
    commands").  Tile's scheduler freely assigns several waits to one
    instruction, so (a) wrap _commit_instruction to hoist extra waits onto
    single-wait NOPs on the same engine just before the offender, and
    (b) split the kernel-tail Drain (which collects one wait per DMA queue)
    into a chain of single-wait Drains."""
    import concourse.tile as tile_mod
    import concourse.mybir as mybir
    import bass_rust
    from concourse.vector_clock import ScopedClock

    if getattr(tile_mod.TileContext, "_single_wait_drain_patch", False):
        return

    orig_commit = tile_mod.TileContext._commit_instruction

    def _commit_instruction(self, inst, lazy_reg_writes=True):
        si = getattr(inst, "sync_info", None)
        if (
            si is not None
            and len(si.on_wait) > 1
            and inst.engine != mybir.EngineType.Unassigned
        ):
            waits = list(si.on_wait)
            for w in waits[:-1]:
                nop = mybir.InstNoOp(
                    name=self.nc.get_next_instruction_name(),
                    engine=inst.engine,
                    sync_info=mybir.SyncInfo(on_wait=[w], on_update=[]),
                    bass_nofuse=True,
                )
                orig_commit(self, nop, lazy_reg_writes=False)
            inst.sync_info = mybir.SyncInfo(
                on_wait=[waits[-1]], on_update=list(si.on_update)
            )
        return orig_commit(self, inst, lazy_reg_writes)

    tile_mod.TileContext._commit_instruction = _commit_instruction

    def _drain_and_barrier(self, tick_clock, wait_clock):
        drain_inst = self.nc.sync.drain()
        wait_clock.add_sem_waits(
            drain_inst.ins, ScopedClock({None: tick_clock.global_clock})
        )
        si = drain_inst.ins.sync_info
        if si is not None and len(si.on_wait) > 1:
            waits = list(si.on_wait)
            drain_inst.ins.sync_info = bass_rust.SyncInfo(
                on_wait=[waits[0]], on_update=list(si.on_update)
            )
            for w in waits[1:]:
                extra = self.nc.sync.drain()
                extra.ins.sync_info = bass_rust.SyncInfo(on_wait=[w], on_update=[])
        self.nc.all_engine_barrier()
        popped = self.nc._tile_sem_poison_stack.pop()
        assert popped is self._sem_poison
        self.nc.clear_and_free_semaphores(list(self.sems.allocated().values()))
        self.nc.all_engine_barrier()

    tile_mod.TileContext._drain_and_barrier = _drain_and_barrier
    tile_mod.TileContext._single_wait_drain_patch = True


def _build():
    global _BUILT
    if _BUILT is not None:
        return _BUILT
    import concourse.bass as bass
    import concourse.tile as tile
    from concourse import mybir

    _patch_tile_drain()

    nc = bass.Bass("TRN2", debug=False)
    # x^T tiled [kt][p(k)][m]: one contiguous [128, 1024] descriptor per
    # k-tile (2KB per partition line).
    xt_d = nc.dram_tensor(
        "xt", [KT, P, M_PER_CORE], mybir.dt.bfloat16, kind="ExternalInput"
    ).ap()
    # n-chunk 0 of w, per-k-tile: [kt][p(k)][n] (1KB lines).
    w0_d = nc.dram_tensor(
        "w0", [KT, P, NW], mybir.dt.bfloat16, kind="ExternalInput"
    ).ap()
    # n-chunks 1-7, baseline tiling [nch][kg][p(k)][ktg][n] (8KB lines).
    wr_d = nc.dram_tensor(
        "wr", [NCH - 1, KG, P, KTG, NW], mybir.dt.bfloat16, kind="ExternalInput"
    ).ap()
    y_d = nc.dram_tensor(
        "y", [M_PER_CORE, OUT], mybir.dt.float32, kind="ExternalOutput"
    ).ap()

    with tile.TileContext(nc) as tc:
        with (
            tc.tile_pool(name="xt", bufs=1) as xt_pool,
            tc.tile_pool(name="w0", bufs=1) as w0_pool,
            tc.tile_pool(name="wr", bufs=2) as wr_pool,
            tc.tile_pool(name="y", bufs=8) as y_pool,
            tc.tile_pool(name="ps", bufs=1, space="PSUM") as ps_pool,
        ):
            xt_sb = xt_pool.tile([P, KT, M_PER_CORE], mybir.dt.bfloat16, tag="xt")
            w0_sbs = []
            # Head: interleave x-slice and w0 descriptors in consumption
            # order so MM(kt=0) waits on just the first two transfers.
            for kt in range(KT):
                nc.sync.dma_start(xt_sb[:, kt, :], xt_d[kt])
                w_sb = w0_pool.tile([P, NW], mybir.dt.bfloat16, name=f"w0_{kt}")
                nc.sync.dma_start(w_sb[:], w0_d[kt])
                w0_sbs.append(w_sb)
            # Prefetch stream for n-chunks 1-7 (pool slots throttle to ~1.5
            # chunks of lookahead).
            wr_sbs = {}
            for nch in range(1, NCH):
                for kg in range(KG):
                    w_sb = wr_pool.tile([P, KTG, NW], mybir.dt.bfloat16, name=f"wr{kg}")
                    nc.sync.dma_start(w_sb[:], wr_d[nch - 1, kg])
                    wr_sbs[(nch, kg)] = w_sb

            def lhsT(mt, kt):
                return xt_sb[:, kt, mt * P : (mt + 1) * P]

            def emit_out(mt, nch, ps, last):
                """psum -> sbuf -> DRAM.  The final group splits work across
                engines/queues to shorten the serial tail."""
                y_sb = y_pool.tile([P, NW], mybir.dt.float32)
                half = NW // 2
                if last:
                    nc.vector.tensor_copy(y_sb[:, :half], ps[:, :half])
                    nc.scalar.copy(y_sb[:, half:], ps[:, half:])
                    for s, eng in ((0, nc.scalar), (1, nc.sync)):
                        eng.dma_start(
                            y_d[
                                mt * P : (mt + 1) * P,
                                nch * NW + s * half : nch * NW + (s + 1) * half,
                            ],
                            y_sb[:, s * half : (s + 1) * half],
                        )
                else:
                    nc.vector.tensor_copy(y_sb[:], ps[:])
                    for s in range(2):
                        nc.scalar.dma_start(
                            y_d[
                                mt * P : (mt + 1) * P,
                                nch * NW + s * half : nch * NW + (s + 1) * half,
                            ],
                            y_sb[:, s * half : (s + 1) * half],
                        )

            # n-chunk 0: k-outer over all 8 m-tiles (8 psum banks live) so
            # each w0[kt] feeds 8 back-to-back MMs (1.73us) while the next
            # k-tile streams in.
            ps0 = [
                ps_pool.tile([P, NW], mybir.dt.float32, name=f"ps0_{m}")
                for m in range(MT)
            ]
            for kt in range(KT):
                for mt in range(MT):
                    nc.tensor.matmul(
                        ps0[mt][:],
                        lhsT=lhsT(mt, kt),
                        rhs=w0_sbs[kt][:],
                        start=(kt == 0),
                        stop=(kt == KT - 1),
                    )
            for mt in range(MT):
                emit_out(mt, 0, ps0[mt], last=False)

            # n-chunks 1-7: m-inner (psum copies stagger across the sweep).
            for nch in range(1, NCH):
                for mt in range(MT):
                    ps = ps_pool.tile([P, NW], mybir.dt.float32, name=f"ps0_{mt}")
                    for kt in range(KT):
                        nc.tensor.matmul(
                            ps[:],
                            lhsT=lhsT(mt, kt),
                            rhs=wr_sbs[(nch, kt // KTG)][:, kt % KTG, :],
                            start=(kt == 0),
                            stop=(kt == KT - 1),
                        )
                    emit_out(
                        mt, nch, ps, last=(nch == NCH - 1 and mt == MT - 1)
                    )
    _BUILT = nc
    return nc


def kernel(x, w_q_fp4, w_os_fp4, w_is_fp4, w_t, w_q_fp8, w_s_fp8):
    global LAST_RESULT
    from concourse.bass_utils import run_bass_kernel_spmd

    x = np.asarray(x, dtype=np.float32)
    w_t = np.asarray(w_t, dtype=np.float32)

    nc = _build()

    xt = np.ascontiguousarray(x.T).astype(ml_dtypes.bfloat16)  # [IN, TOKENS]
    w = w_t.astype(ml_dtypes.bfloat16)
    # n-chunk 0, per-k-tile: [kt, p, n]
    w0 = np.ascontiguousarray(w[:, :NW]).reshape(KT, P, NW)
    # n-chunks 1-7: [k = (kg*KTG + ktg)*P + p, nch*NW + n] -> [nch, kg, p, ktg, n]
    wr = np.ascontiguousarray(
        w[:, NW:].reshape(KG, KTG, P, NCH - 1, NW).transpose(3, 0, 2, 1, 4)
    )
    in_maps = []
    for i in range(NCORES):
        xc = np.ascontiguousarray(
            xt[:, i * M_PER_CORE : (i + 1) * M_PER_CORE]
        ).reshape(KT, P, M_PER_CORE)
        in_maps.append({"xt": xc, "w0": w0, "wr": wr})
    res = None
    for attempt in range(3):
        try:
            res = run_bass_kernel_spmd(
                nc,
                in_maps,
                list(range(NCORES)),
                trace=bool(os.environ.get("BASS_TRACE")),
            )
            break
        except Exception:
            # transient device errors (e.g. NRT_EXEC_UNIT_UNRECOVERABLE)
            # have been observed once and succeeded on retry
            if attempt == 2:
                raise
    LAST_RESULT = res
    return np.concatenate([res.results[i]["y"] for i in range(NCORES)], axis=0)


# revision 8
# speedup vs baseline: 1.1546x; 1.1376x over previous
"""Trainium2 Bass kernel for nn_MixedLinear_89979564851799.

The reference computes y = x @ W.T where W is the block-dequantized weight;
setup_inputs() ships the module's precomputed dequantized transposed weight
w_t (IN, OUT), so y == x @ w_t up to fp32 matmul reassociation.  The kernel
runs a single 8192x4096x4096 matmul, data-parallel over tokens across 8
NeuronCores.

Numerics (v3): mixed bf16 / fp8-DoubleRow.  The last K8 = 256*N8 of the
contraction runs as fp8e4 DoubleRow matmuls (2 k-tiles per MM at the same
216ns issue gap as one bf16 MM -> 2x throughput on that span; measured on
this part, probe_doublerow.py).  The fp8 range covers the module's
fp8-quantized weight partition (k in [3584,4096), whose dequantized values
are EXACTLY representable in TRN fp8e4 under a power-2 scale) plus
256*(N8-2) columns of the fp4 partition (e4m3 rounding error ~2.4% rms on
that slice).  x is e4m3 on the fp8 range.  CPU simulation of the exact
scheme on the reference data: rel err 9.6e-3 (N8=2) / 1.34e-2 (N8=3) /
1.63e-2 (N8=4) vs the 2e-2 gate; bf16-only measures 2.26e-3.

Scale handling: fp8 operands need power-2 scaling (x*2^a, w*2^b) to sit in
e4m3 range; the bf16 operands are pre-scaled by the same powers (exact in
bf16) so both matmul flavors accumulate into one PSUM group, and the
psum->sbuf copy applies 2^-(a+b) (tensor_scalar_mul, same cost as the
plain copy).

Schedule (v2, from the ntff profile of v1): interleaved per-k-tile DMA
descriptors; n-chunk 0 k-OUTER across 8 m-tiles / 8 psum banks (PE starts
~8us in, never starves: demand 222GB/s < ~300GB/s delivered); n-chunks 1-7
m-inner with staggered psum copies; activation table warmed at start; the
final group's copy/store split across engines/queues to shrink the tail.
"""

import os
import numpy as np
import ml_dtypes

P = 128
TOKENS, IN, OUT = 8192, 4096, 4096
NCORES = 8
M_PER_CORE = TOKENS // NCORES      # 1024
KT = IN // P                       # 32 k-tiles
MT = M_PER_CORE // P               # 8 m-tiles
NCH = 8                            # n chunks
NW = OUT // NCH                    # 512 cols per chunk (= 1 PSUM bank fp32)

N8 = 4                             # DoubleRow 256-k blocks (fp8 span = 256*N8)
KTB = KT - 2 * N8                  # bf16 k-tiles
KSPLIT = KTB * P                   # k index where the fp8 span starts
GS = KTB // 2                      # bf16 w chunk size (k-tiles) for nch 1-7

FP8_MAX = 240.0                    # TRN fp8e4 max normal

# Results of the traced run (exec_time_ns etc.) for test harnesses.
LAST_RESULT = None
_BUILT = {}


def _patch_tile_drain():
    """The walrus build in this container rejects instructions carrying more
    than one sync-wait (CoreV3GenImpl setupSyncWait: "Too many sync wait
    commands").  Tile's scheduler freely assigns several waits to one
    instruction, so (a) wrap _commit_instruction to hoist extra waits onto
    single-wait NOPs on the same engine just before the offender, and
    (b) split the kernel-tail Drain (which collects one wait per DMA queue)
    into a chain of single-wait Drains."""
    import concourse.tile as tile_mod
    import concourse.mybir as mybir
    import bass_rust
    from concourse.vector_clock import ScopedClock

    if getattr(tile_mod.TileContext, "_single_wait_drain_patch", False):
        return

    orig_commit = tile_mod.TileContext._commit_instruction

    def _commit_instruction(self, inst, lazy_reg_writes=True):
        si = getattr(inst, "sync_info", None)
        if (
            si is not None
            and len(si.on_wait) > 1
            and inst.engine != mybir.EngineType.Unassigned
        ):
            waits = list(si.on_wait)
            for w in waits[:-1]:
                nop = mybir.InstNoOp(
                    name=self.nc.get_next_instruction_name(),
                    engine=inst.engine,
                    sync_info=mybir.SyncInfo(on_wait=[w], on_update=[]),
                    bass_nofuse=True,
                )
                orig_commit(self, nop, lazy_reg_writes=False)
            inst.sync_info = mybir.SyncInfo(
                on_wait=[waits[-1]], on_update=list(si.on_update)
            )
        return orig_commit(self, inst, lazy_reg_writes)

    tile_mod.TileContext._commit_instruction = _commit_instruction

    def _drain_and_barrier(self, tick_clock, wait_clock):
        drain_inst = self.nc.sync.drain()
        wait_clock.add_sem_waits(
            drain_inst.ins, ScopedClock({None: tick_clock.global_clock})
        )
        si = drain_inst.ins.sync_info
        if si is not None and len(si.on_wait) > 1:
            waits = list(si.on_wait)
            drain_inst.ins.sync_info = bass_rust.SyncInfo(
                on_wait=[waits[0]], on_update=list(si.on_update)
            )
            for w in waits[1:]:
                extra = self.nc.sync.drain()
                extra.ins.sync_info = bass_rust.SyncInfo(on_wait=[w], on_update=[])
        self.nc.all_engine_barrier()
        popped = self.nc._tile_sem_poison_stack.pop()
        assert popped is self._sem_poison
        self.nc.clear_and_free_semaphores(list(self.sems.allocated().values()))
        self.nc.all_engine_barrier()

    tile_mod.TileContext._drain_and_barrier = _drain_and_barrier
    tile_mod.TileContext._single_wait_drain_patch = True


def _build(descale):
    """descale = 2^-(a+b), baked into the psum->sbuf copies."""
    if descale in _BUILT:
        return _BUILT[descale]
    import concourse.bass as bass
    import concourse.tile as tile
    from concourse import mybir

    _patch_tile_drain()

    nc = bass.Bass("TRN2", debug=False)
    xb_d = nc.dram_tensor(
        "xb", [KTB, P, M_PER_CORE], mybir.dt.bfloat16, kind="ExternalInput"
    ).ap()
    x8_d = nc.dram_tensor(
        "x8", [N8, P, 2, M_PER_CORE], mybir.dt.float8e4, kind="ExternalInput"
    ).ap()
    # n-chunk 0 of w, per-k-tile descriptors
    wb0_d = nc.dram_tensor(
        "wb0", [KTB, P, NW], mybir.dt.bfloat16, kind="ExternalInput"
    ).ap()
    w80_d = nc.dram_tensor(
        "w80", [N8, P, 2, NW], mybir.dt.float8e4, kind="ExternalInput"
    ).ap()
    # n-chunks 1-7: bf16 in two GS-k-tile chunks, fp8 in one block
    wbr_d = nc.dram_tensor(
        "wbr", [NCH - 1, 2, P, GS, NW], mybir.dt.bfloat16, kind="ExternalInput"
    ).ap()
    w8r_d = nc.dram_tensor(
        "w8r", [NCH - 1, P, N8, 2, NW], mybir.dt.float8e4, kind="ExternalInput"
    ).ap()
    y_d = nc.dram_tensor(
        "y", [M_PER_CORE, OUT], mybir.dt.float32, kind="ExternalOutput"
    ).ap()

    with tile.TileContext(nc) as tc:
        with (
            tc.tile_pool(name="xt", bufs=1) as xt_pool,
            tc.tile_pool(name="w0", bufs=1) as w0_pool,
            tc.tile_pool(name="wr", bufs=2) as wr_pool,
            tc.tile_pool(name="y", bufs=8) as y_pool,
            tc.tile_pool(name="ps", bufs=1, space="PSUM") as ps_pool,
        ):
            # Warm the activation engine's function table (1.3us, overlaps
            # the DMA head) so the tail's scalar.mul doesn't pay it.
            warm = xt_pool.tile([P, 2], mybir.dt.float32, name="warm")
            nc.scalar.mul(warm[:], warm[:], 0.0)

            xb_sb = xt_pool.tile(
                [P, KTB, M_PER_CORE], mybir.dt.bfloat16, name="xb"
            )
            x8_sb = xt_pool.tile(
                [P, N8, 2, M_PER_CORE], mybir.dt.float8e4, name="x8"
            )
            # Head: interleave x-slice and w0 descriptors in consumption
            # order so MM(kt=0) waits on just the first two transfers.
            wb0_sbs = []
            for kt in range(KTB):
                nc.sync.dma_start(xb_sb[:, kt, :], xb_d[kt])
                w_sb = w0_pool.tile([P, NW], mybir.dt.bfloat16, name=f"wb0_{kt}")
                nc.sync.dma_start(w_sb[:], wb0_d[kt])
                wb0_sbs.append(w_sb)
            w80_sbs = []
            for blk in range(N8):
                nc.sync.dma_start(x8_sb[:, blk], x8_d[blk])
                w_sb = w0_pool.tile([P, 2, NW], mybir.dt.float8e4, name=f"w80_{blk}")
                nc.sync.dma_start(w_sb[:], w80_d[blk])
                w80_sbs.append(w_sb)
            # Prefetch stream for n-chunks 1-7 (pool slots throttle the
            # lookahead to ~1 chunk).
            wbr_sbs = {}
            w8r_sbs = {}
            for nch in range(1, NCH):
                for h in range(2):
                    w_sb = wr_pool.tile(
                        [P, GS, NW], mybir.dt.bfloat16, name=f"wbr{h}"
                    )
                    nc.sync.dma_start(w_sb[:], wbr_d[nch - 1, h])
                    wbr_sbs[(nch, h)] = w_sb
                w_sb = wr_pool.tile(
                    [P, N8, 2, NW], mybir.dt.float8e4, name="w8r"
                )
                nc.sync.dma_start(w_sb[:], w8r_d[nch - 1])
                w8r_sbs[nch] = w_sb

            def mm_group(ps, mt, wb_of_kt, w8_of_blk):
                msl = slice(mt * P, (mt + 1) * P)
                for kt in range(KTB):
                    nc.tensor.matmul(
                        ps[:],
                        lhsT=xb_sb[:, kt, msl],
                        rhs=wb_of_kt(kt),
                        start=(kt == 0),
                        stop=False,
                    )
                for blk in range(N8):
                    nc.tensor.matmul(
                        ps[:],
                        lhsT=x8_sb[:, blk, :, msl],
                        rhs=w8_of_blk(blk),
                        start=False,
                        stop=(blk == N8 - 1),
                        perf_mode=mybir.MatmulPerfMode.DoubleRow,
                    )

            def emit_out(mt, nch, ps, last):
                """psum -> sbuf (descale by 2^-(a+b)) -> DRAM.  The final
                group splits work across engines/queues to shorten the
                serial tail."""
                y_sb = y_pool.tile([P, NW], mybir.dt.float32, name="y_sb")
                half = NW // 2
                if last:
                    nc.vector.tensor_scalar_mul(y_sb[:, :half], ps[:, :half], descale)
                    nc.scalar.mul(y_sb[:, half:], ps[:, half:], descale)
                    for s, eng in ((0, nc.scalar), (1, nc.sync)):
                        eng.dma_start(
                            y_d[
                                mt * P : (mt + 1) * P,
                                nch * NW + s * half : nch * NW + (s + 1) * half,
                            ],
                            y_sb[:, s * half : (s + 1) * half],
                        )
                else:
                    nc.vector.tensor_scalar_mul(y_sb[:], ps[:], descale)
                    for s in range(2):
                        nc.scalar.dma_start(
                            y_d[
                                mt * P : (mt + 1) * P,
                                nch * NW + s * half : nch * NW + (s + 1) * half,
                            ],
                            y_sb[:, s * half : (s + 1) * half],
                        )

            # n-chunk 0: k-outer over all 8 m-tiles (8 psum banks live) so
            # each w tile feeds 8 back-to-back MMs while the next streams in.
            ps0 = [
                ps_pool.tile([P, NW], mybir.dt.float32, name=f"ps0_{m}")
                for m in range(MT)
            ]
            for kt in range(KTB):
                for mt in range(MT):
                    nc.tensor.matmul(
                        ps0[mt][:],
                        lhsT=xb_sb[:, kt, mt * P : (mt + 1) * P],
                        rhs=wb0_sbs[kt][:],
                        start=(kt == 0),
                        stop=False,
                    )
            for blk in range(N8):
                for mt in range(MT):
                    nc.tensor.matmul(
                        ps0[mt][:],
                        lhsT=x8_sb[:, blk, :, mt * P : (mt + 1) * P],
                        rhs=w80_sbs[blk][:],
                        start=False,
                        stop=(blk == N8 - 1),
                        perf_mode=mybir.MatmulPerfMode.DoubleRow,
                    )
            for mt in range(MT):
                emit_out(mt, 0, ps0[mt], last=False)

            # n-chunks 1-7: m-inner (psum copies stagger across the sweep).
            for nch in range(1, NCH):
                for mt in range(MT):
                    ps = ps_pool.tile([P, NW], mybir.dt.float32, name=f"ps0_{mt}")
                    mm_group(
                        ps,
                        mt,
                        lambda kt, n=nch: wbr_sbs[(n, kt // GS)][:, kt % GS, :],
                        lambda blk, n=nch: w8r_sbs[n][:, blk],
                    )
                    emit_out(
                        mt, nch, ps, last=(nch == NCH - 1 and mt == MT - 1)
                    )
    _BUILT[descale] = nc
    return nc


def kernel(x, w_q_fp4, w_os_fp4, w_is_fp4, w_t, w_q_fp8, w_s_fp8):
    global LAST_RESULT
    from concourse.bass_utils import run_bass_kernel_spmd

    x = np.asarray(x, dtype=np.float32)
    w_t = np.asarray(w_t, dtype=np.float32)

    bf16 = ml_dtypes.bfloat16
    e4m3 = ml_dtypes.float8_e4m3  # TRN fp8e4: max normal 240

    # power-2 scales placing the fp8-span operands in e4m3 range
    a = float(np.floor(np.log2(FP8_MAX / np.abs(x).max())))
    b = float(np.floor(np.log2(FP8_MAX / np.abs(w_t[KSPLIT:, :]).max())))
    sa, sb = 2.0**a, 2.0**b
    descale = float(2.0 ** (-(a + b)))

    nc = _build(descale)

    def to8(v, s):
        return np.clip(v * s, -FP8_MAX, FP8_MAX).astype(e4m3)

    xt = np.ascontiguousarray(x.T)                     # [IN, TOKENS] fp32
    xb_all = (xt[:KSPLIT] * sa).astype(bf16)           # [KSPLIT, TOKENS]
    x8_all = to8(xt[KSPLIT:], sa)                      # [2*N8*P, TOKENS]

    wsc = w_t * sb
    # n-chunk 0
    wb0 = np.ascontiguousarray(wsc[:KSPLIT, :NW]).astype(bf16).reshape(KTB, P, NW)
    w80 = np.ascontiguousarray(
        to8(wsc[KSPLIT:, :NW], 1.0).reshape(N8, 2, P, NW).transpose(0, 2, 1, 3)
    )
    # n-chunks 1-7
    wbr = np.ascontiguousarray(
        wsc[:KSPLIT, NW:]
        .astype(bf16)
        .reshape(2, GS, P, NCH - 1, NW)
        .transpose(3, 0, 2, 1, 4)
    )
    w8r = np.ascontiguousarray(
        to8(wsc[KSPLIT:, NW:], 1.0)
        .reshape(N8, 2, P, NCH - 1, NW)
        .transpose(3, 2, 0, 1, 4)
    )
    in_maps = []
    for i in range(NCORES):
        msl = slice(i * M_PER_CORE, (i + 1) * M_PER_CORE)
        xb = np.ascontiguousarray(xb_all[:, msl]).reshape(KTB, P, M_PER_CORE)
        x8 = np.ascontiguousarray(
            x8_all[:, msl].reshape(N8, 2, P, M_PER_CORE).transpose(0, 2, 1, 3)
        )
        in_maps.append(
            {"xb": xb, "x8": x8, "wb0": wb0, "w80": w80, "wbr": wbr, "w8r": w8r}
        )
    res = None
    for attempt in range(3):
        try:
            res = run_bass_kernel_spmd(
                nc,
                in_maps,
                list(range(NCORES)),
                trace=bool(os.environ.get("BASS_TRACE")),
            )
            break
        except Exception:
            # transient device errors (e.g. NRT_EXEC_UNIT_UNRECOVERABLE)
            # have been observed once and succeeded on retry
            if attempt == 2:
                raise
    LAST_RESULT = res
    return np.concatenate([res.results[i]["y"] for i in range(NCORES)], axis=0)
